# revision 10
# baseline (speedup 1.0000x reference)
"""CirConv2d kernel for 8 Trainium2 NeuronCores.

Strategy: data-parallel over batch (2 images per core). The circulant
weight synthesis (softmax-mixed block-circulant projections, ~2.25 MB)
is computed on host in numpy (it is 1.5% of the FLOPs); the 3x3 conv —
the dominant cost — runs on device as 9-tap PSUM-accumulated matmuls
over input-channel tiles, using float32r matmuls (full-rate fp32 path
on the PE for moving dim >= 256).
"""

import sys
import numpy as np

sys.path.insert(0, "/opt/trn_rl_repo")

N_CORES = 8
B, C, H = 16, 256, 56
O, I, KS = 256, 256, 3
BPC = B // N_CORES  # batches per core
SEARCH_SPACE = [1, 2, 4, 8, 16, 32, 64]
GUMBEL_SCALE = 1e-4
TAU = 1.0

HP = H + 2            # padded width 58
NPIX = HP * HP        # 3364
ROWS_PER_CHUNK = 8
NCHUNK = H // ROWS_PER_CHUNK  # 7
NCOL = ROWS_PER_CHUNK * H     # 448 output pixels per matmul

_CACHE = {}


def _synth_weight_host(weight, alphas_after):
    w = alphas_after[0] * weight
    for idx, b in enumerate(SEARCH_SPACE[1:], start=1):
        q, p = O // b, I // b
        tmp = weight.reshape(q, b, p, b, KS, KS).transpose(0, 2, 1, 3, 4, 5)
        ii = np.arange(b)[:, None]
        jj = np.arange(b)[None, :]
        rot = tmp[:, :, ii, (ii + jj) % b]          # q,p,b,b,k,k
        cir = rot.mean(axis=2, dtype=np.float32)     # q,p,b,k,k
        out = cir[:, :, (jj - ii) % b]               # q,p,b,b,k,k
        out = out.transpose(0, 2, 1, 3, 4, 5).reshape(O, I, KS, KS)
        w = w + alphas_after[idx] * out
    return w.astype(np.float32)


def _build(reps_dyn=0):
    import concourse.bacc as bacc
    import concourse.bass as bass
    import concourse.mybir as mybir
    from concourse.tile import TileContext

    AP = bass.AP
    f32 = mybir.dt.float32
    f32r = mybir.dt.float32r

    nc = bacc.Bacc("TRN2", target_bir_lowering=False, debug=False,
                   num_devices=N_CORES)
    xin = nc.declare_dram_parameter("x", [BPC, C, H, H], f32, isOutput=False)
    win = nc.declare_dram_parameter("wsynT", [I, O * 9], f32, isOutput=False)
    yout = nc.declare_dram_parameter("y", [BPC, O, H, H], f32, isOutput=True)

    with TileContext(nc) as tc:
        with tc.tile_pool(name="persist", bufs=1) as pp, \
             tc.tile_pool(name="psum", bufs=4, space="PSUM") as psp, \
             tc.tile_pool(name="load", bufs=2) as ldp, \
             tc.tile_pool(name="stage", bufs=4) as stp:
            # small zero tile used to zero the f32r pad borders
            zt = pp.tile([128, 2 * HP], f32, tag="zt")
            nc.vector.memset(zt[:], 0.0)
            # synthesized weight, transposed: [i, o*9+tap], rounded to f32r
            wt = []
            for it in range(2):
                ws = ldp.tile([128, O * 9], f32, tag="wstage")
                nc.sync.dma_start(out=ws[:], in_=win[it * 128:(it + 1) * 128, :])
                t = pp.tile([128, O * 9], f32r, tag=f"w{it}")
                nc.vector.tensor_copy(t[:], ws[:])
                wt.append(t)
            # zero-padded input images: [b][it] -> [128, 58*58] f32r.
            # x DMA lands contiguous; the pad placement + f32r rounding happen
            # in one DVE copy; borders are zeroed from the f32 zero tile.
            xp = [[None] * 2 for _ in range(BPC)]
            for b in range(BPC):
                for it in range(2):
                    t = pp.tile([128, NPIX], f32r, tag=f"xp{b}{it}")
                    ta = t[:]
                    nc.vector.tensor_copy(
                        AP(ta.tensor, ta.offset, [[NPIX, 128], [1, HP]]),
                        zt[:, 0:HP])
                    nc.vector.tensor_copy(
                        AP(ta.tensor, ta.offset + (HP - 1) * HP,
                           [[NPIX, 128], [1, HP]]),
                        zt[:, 0:HP])
                    nc.vector.tensor_copy(
                        AP(ta.tensor, ta.offset, [[NPIX, 128], [HP, HP], [HP - 1, 2]]),
                        zt[:, 0:2 * HP])
                    xs = ldp.tile([128, H * H], f32, tag="xstage")
                    nc.sync.dma_start(out=xs[:], in_=xin[b, it * 128:(it + 1) * 128, :, :])
                    dst = AP(ta.tensor, ta.offset + HP + 1,
                             [[NPIX, 128], [HP, H], [1, H]])
                    nc.vector.tensor_copy(dst, xs[:])
                    xp[b][it] = t
            def conv_body():
                for b in range(BPC):
                    for ot in range(2):
                        for ch in range(NCHUNK):
                            ps = psp.tile([128, NCOL], f32, tag="ps")
                            idx = 0
                            for it in range(2):
                                wap = wt[it][:]
                                for kh in range(3):
                                    for kw in range(3):
                                        t = kh * 3 + kw
                                        lhsT = AP(wap.tensor,
                                                  wap.offset + ot * 128 * 9 + t,
                                                  [[O * 9, 128], [9, 128]])
                                        xap = xp[b][it][:]
                                        rhs = AP(xap.tensor,
                                                 xap.offset + (ch * ROWS_PER_CHUNK + kh) * HP + kw,
                                                 [[NPIX, 128], [HP, ROWS_PER_CHUNK], [1, H]])
                                        nc.tensor.matmul(ps[:], lhsT, rhs,
                                                         start=(idx == 0),
                                                         stop=(idx == 17))
                                        idx += 1
                            st = stp.tile([128, NCOL], f32, tag="st")
                            nc.scalar.copy(out=st[:], in_=ps[:])
                            ybase = (b * O + ot * 128) * (H * H) + ch * NCOL
                            dst = AP(yout[:].tensor, ybase, [[H * H, 128], [1, NCOL]])
                            nc.sync.dma_start(out=dst, in_=st[:])

            if reps_dyn:
                with tc.For_i(0, reps_dyn, 1):
                    conv_body()
            else:
                conv_body()
    nc.compile()
    return nc


def _get_nc():
    if "nc" not in _CACHE:
        _CACHE["nc"] = _build()
    return _CACHE["nc"]


def _host_prep(x, weight, alphas, gumbels):
    x = np.ascontiguousarray(np.asarray(x, dtype=np.float32))
    weight = np.asarray(weight, dtype=np.float32)
    alphas = np.asarray(alphas, dtype=np.float32)
    gumbels = np.asarray(gumbels, dtype=np.float32)

    a = (alphas + np.float32(GUMBEL_SCALE) * gumbels) / np.float32(TAU)
    a = a - a.max()
    e = np.exp(a, dtype=np.float32)
    alphas_after = (e / e.sum(dtype=np.float32)).astype(np.float32)

    w = _synth_weight_host(weight, alphas_after)  # [O, I, 3, 3]
    wsynT = np.ascontiguousarray(
        w.reshape(O, I, 9).transpose(1, 0, 2).reshape(I, O * 9).astype(np.float32))
    return x, wsynT


def kernel(x, weight, alphas, gumbels):
    x, wsynT = _host_prep(x, weight, alphas, gumbels)
    nc = _get_nc()

    from concourse.bass_utils import run_bass_kernel_spmd
    in_maps = [{"x": x[i * BPC:(i + 1) * BPC], "wsynT": wsynT}
               for i in range(N_CORES)]
    res = run_bass_kernel_spmd(nc, in_maps, list(range(N_CORES)))
    out = np.concatenate([res.results[i]["y"] for i in range(N_CORES)], axis=0)
    return np.ascontiguousarray(out.astype(np.float32))
